# revision 11
# baseline (speedup 1.0000x reference)
"""BitLinear (ternary-quantized linear) Trainium2 kernel.

out = x @ (gamma * ternary(weight)).T + bias, computed tensor-parallel over
8 NeuronCores: weight/bias sharded along out_features, x replicated.

Per-core device program (v6 — no PE transposes, no SWDGE):
  1. x path (per m-tile, pipelined 2 tiles ahead): HWDGE loads the fp32
     rows into SBUF in [128, 2048] halves, the otherwise-idle GPSIMD engine
     casts fp32 -> bf16, and the XBAR DMA-transposes each half straight from
     SBUF into the [K-partition] layout tile. This replaces the previous
     SWDGE casting-DMA stream, whose ~110GB/s descriptor-generation cap
     paced the whole kernel.
  2. The weight shard arrives HOST-TRANSPOSED as wT [K, NS] fp32 (layout-only
     host change; quantization happens on device in fp32, preserving the
     exact ternary boundary). [128 k, 512 n] chunks stream nb-major, loads
     alternating across both HWDGE queues, quantized elementwise to doubled
     ternary {-2,0,2} bf16 straight into the SBUF-resident
     [K-partition, k-subtile, NS] tile wqT:
       even chunks on ACT:  q' = sign(w - thr) + sign(w + thr), add on DVE
       odd  chunks on DVE:  q' = 2*(w >= thr) - 2*(w <= -thr)
     with thr = 0.5*gamma (factor 2 folded into the gamma/2 output scale).
     All supply-side instructions are emitted before any drain so no queue
     ever blocks on a compute-completion semaphore ahead of supply work.
  3. Prologue compute: while the 32MB weight stream is in flight, the PE
     processes m-tiles 0..2 one 512-wide n-block stage at a time, chasing
     the quantizer chunk-by-chunk (3 PSUM banks per stage, so consecutive
     stages double-buffer). Steady state (tiles 3..63) is the per-tile
     kt-outer / nb-inner loop; the last stage and the last tile run
     tile-major so buffers release early / drains overlap matmuls.
  4. Drains: prologue stages evict+bias on DVE with outputs on the Sync
     queue (threaded between steady-state transposes); steady tiles evict
     on ACT (Copy activation, per-partition gamma/2 scale), bias on DVE,
     outputs on the Scalar queue.

gamma = max(mean(|clip(w, -2, 2)|), 1e-4) is a global scalar over the full
weight; it is computed on host with the same jnp ops the module uses so the
quantization boundary matches bit-exactly, and enters the device kernel as a
[128, 4] scalar input tensor (threshold, -threshold, gamma/2).
"""

import numpy as np

import concourse.bass as bass
import concourse.mybir as mybir
import concourse.tile as tile
from concourse import bacc
from concourse.bass_utils import run_bass_kernel_spmd

P = 128
B, S, D_IN, D_OUT = 4, 2048, 4096, 16384
M = B * S                 # 8192 tokens
K = D_IN                  # 4096 contraction
N_CORES = 8
NS = D_OUT // N_CORES     # 2048 out-features per core
KT = K // P               # 32 k-subtiles
MT = M // P               # 64 m-tiles
NBS = 512                 # psum bank free size (fp32)
NB = NS // NBS            # 4 psum n-blocks
KH = K // 2               # x-path half size (2048)
KTH = KT // 2             # k-subtiles per half (16)

F32 = mybir.dt.float32
BF16 = mybir.dt.bfloat16

_NC_CACHE = None
LAST_RESULTS = None


def _build_nc():
    nc = bacc.Bacc(None, target_bir_lowering=False, debug=False)

    x_in = nc.declare_dram_parameter("x", [M, K], F32, isOutput=False)
    w_in = nc.declare_dram_parameter("w", [K, NS], F32, isOutput=False)
    b_in = nc.declare_dram_parameter("bias", [P, NS], F32, isOutput=False)
    s_in = nc.declare_dram_parameter("scal", [P, 4], F32, isOutput=False)
    y_out = nc.declare_dram_parameter("out", [M, NS], F32, isOutput=True)

    PRO_TILES = 3     # m-tiles processed stage-wise during the prologue
    XAHEAD = 2        # steady-state x-path lookahead (bounded by xT bufs)

    with tile.TileContext(nc) as tc:
        with (
            tc.tile_pool(name="const", bufs=1) as constp,
            tc.tile_pool(name="w_sb", bufs=4) as wsbp,
            tc.tile_pool(name="qab", bufs=3) as qabp,
            tc.tile_pool(name="xT", bufs=3) as xTp,
            tc.tile_pool(name="osb", bufs=4) as osbp,
            tc.tile_pool(name="xf32", bufs=2) as xfp,
            tc.tile_pool(name="xbf", bufs=3) as xbp,
            tc.tile_pool(name="psum", bufs=8, space="PSUM") as psump,
        ):
            scal = constp.tile([P, 4], F32)
            nc.sync.dma_start(out=scal[:], in_=s_in[:])
            bias_sb = constp.tile([P, NS], F32)
            nc.sync.dma_start(out=bias_sb[:], in_=b_in[:])
            # full quantized-transposed weight shard, resident in SBUF
            wqT = constp.tile([P, KT, NS], BF16)

            # ---- x path: HWDGE load -> GPSIMD cast -> XBAR transpose ----
            def emit_xload(j, h):
                """Load half h of m-tile j (fp32) and cast to bf16."""
                xf = xfp.tile([P, KH], F32, tag="xf", name=f"xf_{j}_{h}")
                nc.scalar.dma_start(
                    out=xf[:], in_=x_in[j * P:(j + 1) * P, h * KH:(h + 1) * KH]
                )
                xb = xbp.tile([P, KH], BF16, tag="xb", name=f"xb_{j}_{h}")
                nc.gpsimd.tensor_copy(out=xb[:], in_=xf[:])
                return xb

            def emit_xread(j, xT, xbs):
                for h in range(2):
                    nc.sync.dma_start_transpose(
                        xT[:, h * KTH:(h + 1) * KTH, :], xbs[h][:]
                    )

            pro_xTs = [
                xTp.tile([P, KT, P], BF16, tag="xT", name=f"xT_{j}")
                for j in range(PRO_TILES)
            ]
            pro_xbs = {}

            def emit_wload(nb, kt, cidx):
                w_sb = wsbp.tile([P, NBS], F32, tag="w_in")
                dq = nc.sync if (cidx % 2 == 0) else nc.scalar
                dq.dma_start(
                    out=w_sb[:],
                    in_=w_in[kt * P:(kt + 1) * P, nb * NBS:(nb + 1) * NBS],
                )
                return w_sb

            def emit_quant(nb, kt, cidx, w_sb):
                if cidx % 2 == 0:
                    sa = qabp.tile([P, NBS], BF16, tag="q")
                    sb = qabp.tile([P, NBS], BF16, tag="q")
                    nc.scalar.sign(sa[:], w_sb[:], bias=scal[:, 1:2])  # -thr
                    nc.scalar.sign(sb[:], w_sb[:], bias=scal[:, 0:1])  # +thr
                    nc.vector.tensor_tensor(
                        wqT[:, kt, nb * NBS:(nb + 1) * NBS],
                        sa[:], sb[:], mybir.AluOpType.add,
                    )
                else:
                    ga = qabp.tile([P, NBS], BF16, tag="q")
                    gb = qabp.tile([P, NBS], BF16, tag="q")
                    nc.vector.tensor_scalar(
                        ga[:], w_sb[:], scal[:, 0:1], 2.0,
                        mybir.AluOpType.is_ge, mybir.AluOpType.mult,
                    )
                    nc.vector.tensor_scalar(
                        gb[:], w_sb[:], scal[:, 1:2], 2.0,
                        mybir.AluOpType.is_le, mybir.AluOpType.mult,
                    )
                    nc.vector.tensor_tensor(
                        wqT[:, kt, nb * NBS:(nb + 1) * NBS],
                        ga[:], gb[:], mybir.AluOpType.subtract,
                    )

            # x for tile 0 goes out ahead of everything on the scalar queue
            pro_xbs[(0, 0)] = emit_xload(0, 0)
            pro_xbs[(0, 1)] = emit_xload(0, 1)

            # ---- supply emission: all 128 weight chunks, nb-major ----
            # x loads/transposes for prologue tiles are threaded between
            # chunk groups at the points they're needed.
            for nb in range(NB):
                for kt in range(KT):
                    cidx = nb * KT + kt
                    w_sb = emit_wload(nb, kt, cidx)
                    emit_quant(nb, kt, cidx, w_sb)
                    if cidx == 4:
                        emit_xread(0, pro_xTs[0], [pro_xbs[(0, 0)], pro_xbs[(0, 1)]])
                    elif cidx == 12:
                        pro_xbs[(1, 0)] = emit_xload(1, 0)
                        pro_xbs[(1, 1)] = emit_xload(1, 1)
                    elif cidx == 20:
                        emit_xread(1, pro_xTs[1], [pro_xbs[(1, 0)], pro_xbs[(1, 1)]])
                    elif cidx == 28:
                        pro_xbs[(2, 0)] = emit_xload(2, 0)
                        pro_xbs[(2, 1)] = emit_xload(2, 1)
                    elif cidx == 36:
                        emit_xread(2, pro_xTs[2], [pro_xbs[(2, 0)], pro_xbs[(2, 1)]])

            # ---- prologue stages: tiles 0..2 against each n-block ----
            pro_drains = []
            for nb in range(NB):
                pss = [
                    psump.tile([P, NBS], F32, tag="ps", name=f"ps_{j}_{nb}")
                    for j in range(PRO_TILES)
                ]
                if nb in (0, NB - 1):
                    # tile-major: stage 0 because later tiles' x is still
                    # landing; the last stage so xT buffers release early
                    order = [(j, kt) for j in range(PRO_TILES) for kt in range(KT)]
                else:
                    order = [(j, kt) for kt in range(KT) for j in range(PRO_TILES)]
                for j, kt in order:
                    nc.tensor.matmul(
                        pss[j][:],
                        pro_xTs[j][:, kt, :],
                        wqT[:, kt, nb * NBS:(nb + 1) * NBS],
                        start=(kt == 0),
                        stop=(kt == KT - 1),
                    )
                # evict + bias on DVE (keeps the ACT sign stream unblocked);
                # the output DMAs are threaded onto the Sync queue later
                stage_osbs = []
                for j in range(PRO_TILES):
                    osb = osbp.tile([P, NBS], F32, tag="osb", name=f"osb_{j}_{nb}")
                    nc.vector.tensor_scalar(
                        osb[:], pss[j][:], scal[:, 2:3], None,
                        mybir.AluOpType.mult,
                    )
                    nc.vector.tensor_tensor(
                        osb[:], osb[:], bias_sb[:, nb * NBS:(nb + 1) * NBS],
                        mybir.AluOpType.add,
                    )
                    stage_osbs.append((j, osb))
                pro_drains.append(stage_osbs)

            def emit_pro_outs(nb):
                for j, osb in pro_drains[nb]:
                    nc.sync.dma_start(
                        out=y_out[j * P:(j + 1) * P, nb * NBS:(nb + 1) * NBS],
                        in_=osb[:],
                    )

            # ---- steady state: per-tile kt-outer / nb-inner ----
            # x-path lookahead: prime tiles PRO_TILES..PRO_TILES+XAHEAD-1
            pending_xbs = {}
            pending_xTs = {}
            for j in range(PRO_TILES, min(PRO_TILES + XAHEAD, MT)):
                xbs = [emit_xload(j, 0), emit_xload(j, 1)]
                xT = xTp.tile([P, KT, P], BF16, tag="xT", name=f"xT_{j}")
                emit_xread(j, xT, xbs)
                pending_xTs[j] = xT

            for j in range(PRO_TILES, MT):
                # emit the x path for tile j+XAHEAD ahead of this tile's work
                ja = j + XAHEAD
                if ja < MT:
                    xbs = [emit_xload(ja, 0), emit_xload(ja, 1)]
                    xT = xTp.tile([P, KT, P], BF16, tag="xT", name=f"xT_{ja}")
                    emit_xread(ja, xT, xbs)
                    pending_xTs[ja] = xT
                # thread the prologue output DMAs between steady transposes
                if j - PRO_TILES < NB:
                    emit_pro_outs(j - PRO_TILES)
                xT = pending_xTs.pop(j)
                psums = [
                    psump.tile([P, NBS], F32, tag="ps", name=f"ps_{j}_{nb}")
                    for nb in range(NB)
                ]
                last = j == MT - 1
                if last:
                    # nb-outer: drains overlap this tile's own matmuls
                    for nb in range(NB):
                        for kt in range(KT):
                            nc.tensor.matmul(
                                psums[nb][:], xT[:, kt, :],
                                wqT[:, kt, nb * NBS:(nb + 1) * NBS],
                                start=(kt == 0), stop=(kt == KT - 1),
                            )
                        osb = osbp.tile([P, NBS], F32, tag="osb", name=f"osb_{j}_{nb}")
                        nc.scalar.activation(
                            osb[:], psums[nb][:],
                            mybir.ActivationFunctionType.Copy, 0.0, scal[:, 2:3],
                        )
                        nc.vector.tensor_tensor(
                            osb[:], osb[:], bias_sb[:, nb * NBS:(nb + 1) * NBS],
                            mybir.AluOpType.add,
                        )
                        nc.scalar.dma_start(
                            out=y_out[j * P:(j + 1) * P, nb * NBS:(nb + 1) * NBS],
                            in_=osb[:],
                        )
                else:
                    for kt in range(KT):
                        for nb in range(NB):
                            nc.tensor.matmul(
                                psums[nb][:], xT[:, kt, :],
                                wqT[:, kt, nb * NBS:(nb + 1) * NBS],
                                start=(kt == 0), stop=(kt == KT - 1),
                            )
                    for nb in range(NB):
                        osb = osbp.tile([P, NBS], F32, tag="osb", name=f"osb_{j}_{nb}")
                        nc.scalar.activation(
                            osb[:], psums[nb][:],
                            mybir.ActivationFunctionType.Copy, 0.0, scal[:, 2:3],
                        )
                        nc.vector.tensor_tensor(
                            osb[:], osb[:], bias_sb[:, nb * NBS:(nb + 1) * NBS],
                            mybir.AluOpType.add,
                        )
                        nc.scalar.dma_start(
                            out=y_out[j * P:(j + 1) * P, nb * NBS:(nb + 1) * NBS],
                            in_=osb[:],
                        )

    nc.compile()
    return nc


def _compute_gamma(weight: np.ndarray) -> np.float32:
    """Replicate the module's gamma computation bit-exactly (jnp, fp32)."""
    import jax
    import jax.numpy as jnp

    with jax.default_device(jax.devices("cpu")[0]):
        w_f32 = jnp.clip(jnp.asarray(weight, dtype=jnp.float32), -2.0, 2.0)
        gamma = jnp.maximum(jnp.mean(jnp.abs(w_f32)), 1e-4)
        return np.float32(np.asarray(gamma))


def kernel(x: np.ndarray, weight: np.ndarray, bias: np.ndarray) -> np.ndarray:
    global _NC_CACHE, LAST_RESULTS

    x2d = np.ascontiguousarray(np.asarray(x, dtype=np.float32).reshape(M, K))
    weight = np.ascontiguousarray(np.asarray(weight, dtype=np.float32))
    bias = np.asarray(bias, dtype=np.float32)

    gamma = _compute_gamma(weight)
    thr = np.float32(np.float32(0.5) * gamma)
    scal = np.zeros((P, 4), dtype=np.float32)
    scal[:, 0] = thr
    scal[:, 1] = -thr
    scal[:, 2] = np.float32(np.float32(0.5) * gamma)  # psum carries 2x ternary

    if _NC_CACHE is None:
        _NC_CACHE = _build_nc()
    nc = _NC_CACHE

    in_maps = []
    for i in range(N_CORES):
        # host-side layout change only: shard along out_features, then
        # transpose to [K, NS] so device quantization is purely elementwise
        w_shard_T = np.ascontiguousarray(weight[i * NS:(i + 1) * NS].T)
        b_shard = np.ascontiguousarray(
            np.broadcast_to(bias[i * NS:(i + 1) * NS], (P, NS))
        )
        in_maps.append({"x": x2d, "w": w_shard_T, "bias": b_shard, "scal": scal})

    res = run_bass_kernel_spmd(nc, in_maps, list(range(N_CORES)))
    LAST_RESULTS = res

    out = np.concatenate([res.results[i]["out"] for i in range(N_CORES)], axis=1)
    return np.ascontiguousarray(out.reshape(B, S, D_OUT))


# revision 12
# speedup vs baseline: 1.0779x; 1.0779x over previous
"""BitLinear (ternary-quantized linear) Trainium2 kernel.

out = x @ (gamma * ternary(weight)).T + bias, computed tensor-parallel over
8 NeuronCores: weight/bias sharded along out_features, x replicated.

Per-core device program (v7):
  1. x path: every m-tile is cast fp32 -> bf16 into a DRAM scratch tile and
     then XBAR DMA-transposed DRAM->SBUF into [K-partition] layout. Most
     tiles cast via SWDGE DRAM->DRAM DMAs (throttled behind consumption),
     but SWDGE descriptor generation caps at ~110GB/s (~30us/tile) — just
     above the 27.3us/tile PE floor — so the three prologue tiles and every
     4th steady tile instead take a fast path: HWDGE fp32 load -> DVE cast
     -> HWDGE bf16 store. That drops SWDGE to 45 tiles (~1350us), well off
     the critical path.
  2. The weight shard arrives HOST-TRANSPOSED as wT [K, NS] fp32 (layout-only
     host change; quantization happens on device in fp32, preserving the
     exact ternary boundary). [128 k, 512 n] chunks stream nb-major, loads
     alternating across both HWDGE queues, quantized elementwise to doubled
     ternary {-2,0,2} bf16 straight into the SBUF-resident
     [K-partition, k-subtile, NS] tile wqT:
       even chunks on ACT:  q' = sign(w - thr) + sign(w + thr), add on DVE
       odd  chunks on DVE:  q' = 2*(w >= thr) - 2*(w <= -thr)
     with thr = 0.5*gamma (factor 2 folded into the gamma/2 output scale).
     All supply-side instructions are emitted before any drain so no queue
     ever blocks on a compute-completion semaphore ahead of supply work.
  3. Prologue compute: while the 32MB weight stream is in flight, the PE
     processes m-tiles 0..2 one 512-wide n-block stage at a time, chasing
     the quantizer chunk-by-chunk (3 PSUM banks per stage, so consecutive
     stages double-buffer). Steady state (tiles 3..63) is the per-tile
     kt-outer / nb-inner loop; the first and last stages and the last tile
     run tile-major so buffers release early / drains overlap matmuls.
  4. Drains: prologue stages evict+bias on DVE with outputs on the Sync
     queue (threaded between steady-state transposes); steady tiles evict
     on ACT (Copy activation, per-partition gamma/2 scale), bias on DVE,
     outputs on the Scalar queue.

gamma = max(mean(|clip(w, -2, 2)|), 1e-4) is a global scalar over the full
weight; it is computed on host with the same jnp ops the module uses so the
quantization boundary matches bit-exactly, and enters the device kernel as a
[128, 4] scalar input tensor (threshold, -threshold, gamma/2).
"""

import numpy as np

import concourse.bass as bass
import concourse.mybir as mybir
import concourse.tile as tile
from concourse import bacc
from concourse.bass_utils import run_bass_kernel_spmd
from concourse.tile import add_dep_helper

P = 128
B, S, D_IN, D_OUT = 4, 2048, 4096, 16384
M = B * S                 # 8192 tokens
K = D_IN                  # 4096 contraction
N_CORES = 8
NS = D_OUT // N_CORES     # 2048 out-features per core
KT = K // P               # 32 k-subtiles
MT = M // P               # 64 m-tiles
NBS = 512                 # psum bank free size (fp32)
NB = NS // NBS            # 4 psum n-blocks
KQ = K // 4               # x fast-path quarter size (1024)
KTQ = KT // 4             # k-subtiles per quarter (8)

F32 = mybir.dt.float32
BF16 = mybir.dt.bfloat16

_NC_CACHE = None
LAST_RESULTS = None

PRO_TILES = 3


def _is_fast(j):
    """Tiles cast via the HWDGE+DVE fast path instead of SWDGE."""
    return j < PRO_TILES or j % 4 == 3


def _build_nc():
    nc = bacc.Bacc(None, target_bir_lowering=False, debug=False)

    x_in = nc.declare_dram_parameter("x", [M, K], F32, isOutput=False)
    w_in = nc.declare_dram_parameter("w", [K, NS], F32, isOutput=False)
    b_in = nc.declare_dram_parameter("bias", [P, NS], F32, isOutput=False)
    s_in = nc.declare_dram_parameter("scal", [P, 4], F32, isOutput=False)
    y_out = nc.declare_dram_parameter("out", [M, NS], F32, isOutput=True)

    CAST_AHEAD = 6    # SWDGE cast throttle distance (in m-tiles)

    with tile.TileContext(nc) as tc:
        with (
            tc.tile_pool(name="const", bufs=1) as constp,
            tc.tile_pool(name="w_sb", bufs=4) as wsbp,
            tc.tile_pool(name="qab", bufs=3) as qabp,
            tc.tile_pool(name="xT", bufs=4) as xTp,
            tc.tile_pool(name="osb", bufs=4) as osbp,
            tc.tile_pool(name="xf32", bufs=2) as xfp,
            tc.tile_pool(name="xbf", bufs=2) as xbp,
            tc.tile_pool(name="psum", bufs=8, space="PSUM") as psump,
            tc.tile_pool(name="dram", bufs=1, space="DRAM") as dramp,
        ):
            scal = constp.tile([P, 4], F32)
            nc.sync.dma_start(out=scal[:], in_=s_in[:])
            bias_sb = constp.tile([P, NS], F32)
            nc.sync.dma_start(out=bias_sb[:], in_=b_in[:])
            # full quantized-transposed weight shard, resident in SBUF
            wqT = constp.tile([P, KT, NS], BF16)

            # DRAM bf16 scratch for every m-tile
            xhat = [dramp.tile([P, K], BF16, name=f"xhat_{j}") for j in range(MT)]

            # SWDGE casts for the slow-path tiles, emitted upfront; the
            # first few are gated on weight-stream progress, later ones are
            # throttled against consumption via xread deps.
            cast_insts = {}
            swdge_tiles = [j for j in range(MT) if not _is_fast(j)]
            for j in swdge_tiles:
                cast_insts[j] = nc.gpsimd.dma_start(
                    out=xhat[j][:], in_=x_in[j * P:(j + 1) * P, :]
                )

            def emit_fast_x(j):
                """HWDGE fp32 load -> DVE cast -> HWDGE bf16 store, quarters."""
                for q in range(4):
                    xf = xfp.tile([P, KQ], F32, tag="xf", name=f"xf_{j}_{q}")
                    nc.scalar.dma_start(
                        out=xf[:], in_=x_in[j * P:(j + 1) * P, q * KQ:(q + 1) * KQ]
                    )
                    xb = xbp.tile([P, KQ], BF16, tag="xb", name=f"xb_{j}_{q}")
                    nc.vector.tensor_copy(out=xb[:], in_=xf[:])
                    nc.scalar.dma_start(
                        out=xhat[j][:, q * KQ:(q + 1) * KQ], in_=xb[:]
                    )

            def emit_xread(j, xT, quarters=False):
                if quarters:
                    xr = None
                    for q in range(4):
                        xr = nc.sync.dma_start_transpose(
                            xT[:, q * KTQ:(q + 1) * KTQ, :],
                            xhat[j][:, q * KQ:(q + 1) * KQ],
                        )
                else:
                    xr = nc.sync.dma_start_transpose(xT[:], xhat[j][:])
                ja = j + CAST_AHEAD
                if ja in cast_insts:
                    add_dep_helper(
                        cast_insts[ja].ins,
                        xr.ins,
                        reason="throttle x-cast to stay a few m-tiles ahead",
                    )
                return xr

            pro_xTs = [
                xTp.tile([P, KT, P], BF16, tag="xT", name=f"xT_{j}")
                for j in range(PRO_TILES)
            ]

            def emit_wload(nb, kt, cidx):
                w_sb = wsbp.tile([P, NBS], F32, tag="w_in")
                dq = nc.sync if (cidx % 2 == 0) else nc.scalar
                wl = dq.dma_start(
                    out=w_sb[:],
                    in_=w_in[kt * P:(kt + 1) * P, nb * NBS:(nb + 1) * NBS],
                )
                return w_sb, wl

            def emit_quant(nb, kt, cidx, w_sb):
                if cidx % 2 == 0:
                    sa = qabp.tile([P, NBS], BF16, tag="q")
                    sb = qabp.tile([P, NBS], BF16, tag="q")
                    nc.scalar.sign(sa[:], w_sb[:], bias=scal[:, 1:2])  # -thr
                    nc.scalar.sign(sb[:], w_sb[:], bias=scal[:, 0:1])  # +thr
                    nc.vector.tensor_tensor(
                        wqT[:, kt, nb * NBS:(nb + 1) * NBS],
                        sa[:], sb[:], mybir.AluOpType.add,
                    )
                else:
                    ga = qabp.tile([P, NBS], BF16, tag="q")
                    gb = qabp.tile([P, NBS], BF16, tag="q")
                    nc.vector.tensor_scalar(
                        ga[:], w_sb[:], scal[:, 0:1], 2.0,
                        mybir.AluOpType.is_ge, mybir.AluOpType.mult,
                    )
                    nc.vector.tensor_scalar(
                        gb[:], w_sb[:], scal[:, 1:2], 2.0,
                        mybir.AluOpType.is_le, mybir.AluOpType.mult,
                    )
                    nc.vector.tensor_tensor(
                        wqT[:, kt, nb * NBS:(nb + 1) * NBS],
                        ga[:], gb[:], mybir.AluOpType.subtract,
                    )

            # x for tile 0 goes out ahead of the weight odds on scalar
            emit_fast_x(0)

            # ---- supply emission: all 128 weight chunks, nb-major ----
            for nb in range(NB):
                for kt in range(KT):
                    cidx = nb * KT + kt
                    w_sb, wl = emit_wload(nb, kt, cidx)
                    emit_quant(nb, kt, cidx, w_sb)
                    if cidx == 4:
                        emit_xread(0, pro_xTs[0], quarters=True)
                    elif cidx == 12:
                        emit_fast_x(1)
                    elif cidx == 20:
                        emit_xread(1, pro_xTs[1])
                    elif cidx == 28:
                        emit_fast_x(2)
                    elif cidx == 36:
                        emit_xread(2, pro_xTs[2])
                    # release the first SWDGE casts as the weight stream
                    # winds down
                    elif cidx == 64 and 4 in cast_insts:
                        add_dep_helper(cast_insts[4].ins, wl.ins,
                                       reason="prologue: weights own HBM")
                    elif cidx == 96 and 5 in cast_insts:
                        add_dep_helper(cast_insts[5].ins, wl.ins,
                                       reason="prologue: weights own HBM")
                    elif cidx == 127 and 6 in cast_insts:
                        add_dep_helper(cast_insts[6].ins, wl.ins,
                                       reason="prologue: weights own HBM")

            # ---- prologue stages: tiles 0..2 against each n-block ----
            pro_drains = []
            for nb in range(NB):
                pss = [
                    psump.tile([P, NBS], F32, tag="ps", name=f"ps_{j}_{nb}")
                    for j in range(PRO_TILES)
                ]
                if nb in (0, NB - 1):
                    order = [(j, kt) for j in range(PRO_TILES) for kt in range(KT)]
                else:
                    order = [(j, kt) for kt in range(KT) for j in range(PRO_TILES)]
                for j, kt in order:
                    nc.tensor.matmul(
                        pss[j][:],
                        pro_xTs[j][:, kt, :],
                        wqT[:, kt, nb * NBS:(nb + 1) * NBS],
                        start=(kt == 0),
                        stop=(kt == KT - 1),
                    )
                stage_osbs = []
                for j in range(PRO_TILES):
                    osb = osbp.tile([P, NBS], F32, tag="osb", name=f"osb_{j}_{nb}")
                    nc.vector.tensor_scalar(
                        osb[:], pss[j][:], scal[:, 2:3], None,
                        mybir.AluOpType.mult,
                    )
                    nc.vector.tensor_tensor(
                        osb[:], osb[:], bias_sb[:, nb * NBS:(nb + 1) * NBS],
                        mybir.AluOpType.add,
                    )
                    stage_osbs.append((j, osb))
                pro_drains.append(stage_osbs)

            def emit_pro_outs(nb):
                for j, osb in pro_drains[nb]:
                    nc.sync.dma_start(
                        out=y_out[j * P:(j + 1) * P, nb * NBS:(nb + 1) * NBS],
                        in_=osb[:],
                    )

            # ---- steady state: per-tile kt-outer / nb-inner ----
            XAHEAD = 2
            pending_xTs = {}

            def prime_x(ja):
                if _is_fast(ja) and ja >= PRO_TILES:
                    emit_fast_x(ja)
                xT = xTp.tile([P, KT, P], BF16, tag="xT", name=f"xT_{ja}")
                emit_xread(ja, xT)
                pending_xTs[ja] = xT

            for ja in range(PRO_TILES, min(PRO_TILES + XAHEAD, MT)):
                prime_x(ja)

            for j in range(PRO_TILES, MT):
                if j + XAHEAD < MT:
                    prime_x(j + XAHEAD)
                if j - PRO_TILES < NB:
                    emit_pro_outs(j - PRO_TILES)
                xT = pending_xTs.pop(j)
                psums = [
                    psump.tile([P, NBS], F32, tag="ps", name=f"ps_{j}_{nb}")
                    for nb in range(NB)
                ]
                last = j == MT - 1
                if last:
                    for nb in range(NB):
                        for kt in range(KT):
                            nc.tensor.matmul(
                                psums[nb][:], xT[:, kt, :],
                                wqT[:, kt, nb * NBS:(nb + 1) * NBS],
                                start=(kt == 0), stop=(kt == KT - 1),
                            )
                        osb = osbp.tile([P, NBS], F32, tag="osb", name=f"osb_{j}_{nb}")
                        nc.scalar.activation(
                            osb[:], psums[nb][:],
                            mybir.ActivationFunctionType.Copy, 0.0, scal[:, 2:3],
                        )
                        nc.vector.tensor_tensor(
                            osb[:], osb[:], bias_sb[:, nb * NBS:(nb + 1) * NBS],
                            mybir.AluOpType.add,
                        )
                        nc.scalar.dma_start(
                            out=y_out[j * P:(j + 1) * P, nb * NBS:(nb + 1) * NBS],
                            in_=osb[:],
                        )
                else:
                    for kt in range(KT):
                        for nb in range(NB):
                            nc.tensor.matmul(
                                psums[nb][:], xT[:, kt, :],
                                wqT[:, kt, nb * NBS:(nb + 1) * NBS],
                                start=(kt == 0), stop=(kt == KT - 1),
                            )
                    for nb in range(NB):
                        osb = osbp.tile([P, NBS], F32, tag="osb", name=f"osb_{j}_{nb}")
                        nc.scalar.activation(
                            osb[:], psums[nb][:],
                            mybir.ActivationFunctionType.Copy, 0.0, scal[:, 2:3],
                        )
                        nc.vector.tensor_tensor(
                            osb[:], osb[:], bias_sb[:, nb * NBS:(nb + 1) * NBS],
                            mybir.AluOpType.add,
                        )
                        nc.scalar.dma_start(
                            out=y_out[j * P:(j + 1) * P, nb * NBS:(nb + 1) * NBS],
                            in_=osb[:],
                        )

    nc.compile()
    return nc


def _compute_gamma(weight: np.ndarray) -> np.float32:
    """Replicate the module's gamma computation bit-exactly (jnp, fp32)."""
    import jax
    import jax.numpy as jnp

    with jax.default_device(jax.devices("cpu")[0]):
        w_f32 = jnp.clip(jnp.asarray(weight, dtype=jnp.float32), -2.0, 2.0)
        gamma = jnp.maximum(jnp.mean(jnp.abs(w_f32)), 1e-4)
        return np.float32(np.asarray(gamma))


def kernel(x: np.ndarray, weight: np.ndarray, bias: np.ndarray) -> np.ndarray:
    global _NC_CACHE, LAST_RESULTS

    x2d = np.ascontiguousarray(np.asarray(x, dtype=np.float32).reshape(M, K))
    weight = np.ascontiguousarray(np.asarray(weight, dtype=np.float32))
    bias = np.asarray(bias, dtype=np.float32)

    gamma = _compute_gamma(weight)
    thr = np.float32(np.float32(0.5) * gamma)
    scal = np.zeros((P, 4), dtype=np.float32)
    scal[:, 0] = thr
    scal[:, 1] = -thr
    scal[:, 2] = np.float32(np.float32(0.5) * gamma)  # psum carries 2x ternary

    if _NC_CACHE is None:
        _NC_CACHE = _build_nc()
    nc = _NC_CACHE

    in_maps = []
    for i in range(N_CORES):
        # host-side layout change only: shard along out_features, then
        # transpose to [K, NS] so device quantization is purely elementwise
        w_shard_T = np.ascontiguousarray(weight[i * NS:(i + 1) * NS].T)
        b_shard = np.ascontiguousarray(
            np.broadcast_to(bias[i * NS:(i + 1) * NS], (P, NS))
        )
        in_maps.append({"x": x2d, "w": w_shard_T, "bias": b_shard, "scal": scal})

    res = run_bass_kernel_spmd(nc, in_maps, list(range(N_CORES)))
    LAST_RESULTS = res

    out = np.concatenate([res.results[i]["out"] for i in range(N_CORES)], axis=1)
    return np.ascontiguousarray(out.reshape(B, S, D_OUT))


# revision 15
# speedup vs baseline: 1.1355x; 1.0535x over previous
"""BitLinear (ternary-quantized linear) Trainium2 kernel.

out = x @ (gamma * ternary(weight)).T + bias, computed tensor-parallel over
8 NeuronCores: weight/bias sharded along out_features, x replicated.

Per-core device program (v7):
  1. x path: every m-tile is cast fp32 -> bf16 into a DRAM scratch tile and
     then XBAR DMA-transposed DRAM->SBUF into [K-partition] layout. Most
     tiles cast via SWDGE DRAM->DRAM DMAs (throttled behind consumption),
     but SWDGE descriptor generation caps at ~110GB/s (~30us/tile) — just
     above the 27.3us/tile PE floor — so the three prologue tiles and every
     4th steady tile instead take a fast path: HWDGE fp32 load -> DVE cast
     -> HWDGE bf16 store. That drops SWDGE to 45 tiles (~1350us), well off
     the critical path.
  2. The weight shard arrives HOST-TRANSPOSED as wT [K, NS] fp32 (layout-only
     host change; quantization happens on device in fp32, preserving the
     exact ternary boundary). [128 k, 512 n] chunks stream nb-major, loads
     alternating across both HWDGE queues, quantized elementwise to doubled
     ternary {-2,0,2} bf16 straight into the SBUF-resident
     [K-partition, k-subtile, NS] tile wqT:
       even chunks on ACT:  q' = sign(w - thr) + sign(w + thr), add on DVE
       odd  chunks on DVE:  q' = 2*(w >= thr) - 2*(w <= -thr)
     with thr = 0.5*gamma (factor 2 folded into the gamma/2 output scale).
     All supply-side instructions are emitted before any drain so no queue
     ever blocks on a compute-completion semaphore ahead of supply work.
  3. Prologue compute: while the 32MB weight stream is in flight, the PE
     processes m-tiles 0..2 one 512-wide n-block stage at a time, chasing
     the quantizer chunk-by-chunk (3 PSUM banks per stage, so consecutive
     stages double-buffer). Steady state (tiles 3..63) is the per-tile
     kt-outer / nb-inner loop; the first and last stages and the last tile
     run tile-major so buffers release early / drains overlap matmuls.
  4. Drains: prologue stages evict+bias on DVE with outputs on the Sync
     queue (threaded between steady-state transposes); steady tiles evict
     on ACT (Copy activation, per-partition gamma/2 scale), bias on DVE,
     outputs on the Scalar queue.

gamma = max(mean(|clip(w, -2, 2)|), 1e-4) is a global scalar over the full
weight; it is computed on host with the same jnp ops the module uses so the
quantization boundary matches bit-exactly, and enters the device kernel as a
[128, 4] scalar input tensor (threshold, -threshold, gamma/2).
"""

import numpy as np

import concourse.bass as bass
import concourse.mybir as mybir
import concourse.tile as tile
from concourse import bacc
from concourse.bass_utils import run_bass_kernel_spmd
from concourse.tile import add_dep_helper

P = 128
B, S, D_IN, D_OUT = 4, 2048, 4096, 16384
M = B * S                 # 8192 tokens
K = D_IN                  # 4096 contraction
N_CORES = 8
NS = D_OUT // N_CORES     # 2048 out-features per core
KT = K // P               # 32 k-subtiles
MT = M // P               # 64 m-tiles
NBS = 512                 # psum bank free size (fp32)
NB = NS // NBS            # 4 psum n-blocks
KQ = K // 4               # x fast-path quarter size (1024)
KTQ = KT // 4             # k-subtiles per quarter (8)

F32 = mybir.dt.float32
BF16 = mybir.dt.bfloat16

_NC_CACHE = None
LAST_RESULTS = None

PRO_TILES = 3


def _is_fast(j):
    """Tiles cast via the HWDGE+DVE fast path instead of SWDGE."""
    return j < PRO_TILES


def _build_nc():
    nc = bacc.Bacc(None, target_bir_lowering=False, debug=False)

    x_in = nc.declare_dram_parameter("x", [M, K], F32, isOutput=False)
    w_in = nc.declare_dram_parameter("w", [K, NS], F32, isOutput=False)
    b_in = nc.declare_dram_parameter("bias", [P, NS], F32, isOutput=False)
    s_in = nc.declare_dram_parameter("scal", [P, 4], F32, isOutput=False)
    y_out = nc.declare_dram_parameter("out", [M, NS], F32, isOutput=True)

    CAST_AHEAD = 6    # SWDGE cast throttle distance (in m-tiles)

    with tile.TileContext(nc) as tc:
        with (
            tc.tile_pool(name="const", bufs=1) as constp,
            tc.tile_pool(name="w_sb", bufs=4) as wsbp,
            tc.tile_pool(name="qab", bufs=3) as qabp,
            tc.tile_pool(name="xT", bufs=4) as xTp,
            tc.tile_pool(name="osb", bufs=4) as osbp,
            tc.tile_pool(name="xf32", bufs=2) as xfp,
            tc.tile_pool(name="xbf", bufs=2) as xbp,
            tc.tile_pool(name="psum", bufs=8, space="PSUM") as psump,
            tc.tile_pool(name="dram", bufs=1, space="DRAM") as dramp,
        ):
            scal = constp.tile([P, 4], F32)
            nc.sync.dma_start(out=scal[:], in_=s_in[:])
            bias_sb = constp.tile([P, NS], F32)
            nc.sync.dma_start(out=bias_sb[:], in_=b_in[:])
            # full quantized-transposed weight shard, resident in SBUF
            wqT = constp.tile([P, KT, NS], BF16)

            # DRAM bf16 scratch for every m-tile
            xhat = [dramp.tile([P, K], BF16, name=f"xhat_{j}") for j in range(MT)]

            # SWDGE casts for the slow-path tiles, emitted upfront; the
            # first few are gated on weight-stream progress, later ones are
            # throttled against consumption via xread deps.
            cast_insts = {}
            swdge_tiles = [j for j in range(MT) if not _is_fast(j)]
            for j in swdge_tiles:
                cast_insts[j] = nc.gpsimd.dma_start(
                    out=xhat[j][:], in_=x_in[j * P:(j + 1) * P, :]
                )

            def emit_fast_x(j):
                """HWDGE fp32 load -> DVE cast -> HWDGE bf16 store, quarters."""
                for q in range(4):
                    xf = xfp.tile([P, KQ], F32, tag="xf", name=f"xf_{j}_{q}")
                    nc.scalar.dma_start(
                        out=xf[:], in_=x_in[j * P:(j + 1) * P, q * KQ:(q + 1) * KQ]
                    )
                    xb = xbp.tile([P, KQ], BF16, tag="xb", name=f"xb_{j}_{q}")
                    nc.vector.tensor_copy(out=xb[:], in_=xf[:])
                    nc.scalar.dma_start(
                        out=xhat[j][:, q * KQ:(q + 1) * KQ], in_=xb[:]
                    )

            def emit_xread(j, xT, quarters=False):
                if quarters:
                    xr = None
                    for q in range(4):
                        xr = nc.sync.dma_start_transpose(
                            xT[:, q * KTQ:(q + 1) * KTQ, :],
                            xhat[j][:, q * KQ:(q + 1) * KQ],
                        )
                else:
                    xr = nc.sync.dma_start_transpose(xT[:], xhat[j][:])
                ja = j + CAST_AHEAD
                if ja in cast_insts and ja > 8:
                    add_dep_helper(
                        cast_insts[ja].ins,
                        xr.ins,
                        reason="throttle x-cast to stay a few m-tiles ahead",
                    )
                return xr

            pro_xTs = [
                xTp.tile([P, KT, P], BF16, tag="xT", name=f"xT_{j}")
                for j in range(PRO_TILES)
            ]

            def emit_wload(nb, kt, cidx):
                w_sb = wsbp.tile([P, NBS], F32, tag="w_in")
                dq = nc.sync if (cidx % 2 == 0) else nc.scalar
                wl = dq.dma_start(
                    out=w_sb[:],
                    in_=w_in[kt * P:(kt + 1) * P, nb * NBS:(nb + 1) * NBS],
                )
                return w_sb, wl

            def emit_quant(nb, kt, cidx, w_sb):
                if cidx % 2 == 0:
                    sa = qabp.tile([P, NBS], BF16, tag="q")
                    sb = qabp.tile([P, NBS], BF16, tag="q")
                    nc.scalar.sign(sa[:], w_sb[:], bias=scal[:, 1:2])  # -thr
                    nc.scalar.sign(sb[:], w_sb[:], bias=scal[:, 0:1])  # +thr
                    nc.vector.tensor_tensor(
                        wqT[:, kt, nb * NBS:(nb + 1) * NBS],
                        sa[:], sb[:], mybir.AluOpType.add,
                    )
                else:
                    ga = qabp.tile([P, NBS], BF16, tag="q")
                    gb = qabp.tile([P, NBS], BF16, tag="q")
                    nc.vector.tensor_scalar(
                        ga[:], w_sb[:], scal[:, 0:1], 2.0,
                        mybir.AluOpType.is_ge, mybir.AluOpType.mult,
                    )
                    nc.vector.tensor_scalar(
                        gb[:], w_sb[:], scal[:, 1:2], 2.0,
                        mybir.AluOpType.is_le, mybir.AluOpType.mult,
                    )
                    nc.vector.tensor_tensor(
                        wqT[:, kt, nb * NBS:(nb + 1) * NBS],
                        ga[:], gb[:], mybir.AluOpType.subtract,
                    )

            # x for tile 0 goes out ahead of the weight odds on scalar
            emit_fast_x(0)

            # ---- supply emission: all 128 weight chunks, nb-major ----
            for nb in range(NB):
                for kt in range(KT):
                    cidx = nb * KT + kt
                    w_sb, wl = emit_wload(nb, kt, cidx)
                    emit_quant(nb, kt, cidx, w_sb)
                    if cidx == 4:
                        emit_xread(0, pro_xTs[0], quarters=True)
                    elif cidx == 12:
                        emit_fast_x(1)
                    elif cidx == 20:
                        emit_xread(1, pro_xTs[1])
                    elif cidx == 28:
                        emit_fast_x(2)
                    elif cidx == 36:
                        emit_xread(2, pro_xTs[2])
                    # release the first SWDGE casts as the weight stream
                    # winds down (cast 3 runs free; 4 at ~2/3 through; 5-8
                    # as soon as the last chunk is queued so the cast
                    # buffer builds before steady state)
                    elif cidx == 80 and 4 in cast_insts:
                        add_dep_helper(cast_insts[4].ins, wl.ins,
                                       reason="prologue: weights own HBM")
                    elif cidx == 127:
                        for cj in (5, 6, 7, 8):
                            if cj in cast_insts:
                                add_dep_helper(cast_insts[cj].ins, wl.ins,
                                               reason="prologue: weights own HBM")

            # ---- prologue stages: tiles 0..2 against each n-block ----
            pro_drains = []
            for nb in range(NB):
                pss = [
                    psump.tile([P, NBS], F32, tag="ps", name=f"ps_{j}_{nb}")
                    for j in range(PRO_TILES)
                ]
                if nb in (0, NB - 1):
                    order = [(j, kt) for j in range(PRO_TILES) for kt in range(KT)]
                else:
                    order = [(j, kt) for kt in range(KT) for j in range(PRO_TILES)]
                for j, kt in order:
                    nc.tensor.matmul(
                        pss[j][:],
                        pro_xTs[j][:, kt, :],
                        wqT[:, kt, nb * NBS:(nb + 1) * NBS],
                        start=(kt == 0),
                        stop=(kt == KT - 1),
                    )
                stage_osbs = []
                for j in range(PRO_TILES):
                    osb = osbp.tile([P, NBS], F32, tag="osb", name=f"osb_{j}_{nb}")
                    nc.vector.tensor_scalar(
                        osb[:], pss[j][:], scal[:, 2:3], None,
                        mybir.AluOpType.mult,
                    )
                    nc.vector.tensor_tensor(
                        osb[:], osb[:], bias_sb[:, nb * NBS:(nb + 1) * NBS],
                        mybir.AluOpType.add,
                    )
                    stage_osbs.append((j, osb))
                pro_drains.append(stage_osbs)

            def emit_pro_outs(nb):
                for j, osb in pro_drains[nb]:
                    nc.sync.dma_start(
                        out=y_out[j * P:(j + 1) * P, nb * NBS:(nb + 1) * NBS],
                        in_=osb[:],
                    )

            # ---- steady state: per-tile kt-outer / nb-inner ----
            XAHEAD = 2
            pending_xTs = {}

            def prime_x(ja):
                if _is_fast(ja) and ja >= PRO_TILES:
                    emit_fast_x(ja)
                xT = xTp.tile([P, KT, P], BF16, tag="xT", name=f"xT_{ja}")
                emit_xread(ja, xT)
                pending_xTs[ja] = xT

            for ja in range(PRO_TILES, min(PRO_TILES + XAHEAD, MT)):
                prime_x(ja)

            for j in range(PRO_TILES, MT):
                if j + XAHEAD < MT:
                    prime_x(j + XAHEAD)
                if j - PRO_TILES < NB:
                    emit_pro_outs(j - PRO_TILES)
                xT = pending_xTs.pop(j)
                psums = [
                    psump.tile([P, NBS], F32, tag="ps", name=f"ps_{j}_{nb}")
                    for nb in range(NB)
                ]
                last = j == MT - 1
                if last:
                    for nb in range(NB):
                        for kt in range(KT):
                            nc.tensor.matmul(
                                psums[nb][:], xT[:, kt, :],
                                wqT[:, kt, nb * NBS:(nb + 1) * NBS],
                                start=(kt == 0), stop=(kt == KT - 1),
                            )
                        osb = osbp.tile([P, NBS], F32, tag="osb", name=f"osb_{j}_{nb}")
                        nc.scalar.activation(
                            osb[:], psums[nb][:],
                            mybir.ActivationFunctionType.Copy, 0.0, scal[:, 2:3],
                        )
                        nc.vector.tensor_tensor(
                            osb[:], osb[:], bias_sb[:, nb * NBS:(nb + 1) * NBS],
                            mybir.AluOpType.add,
                        )
                        nc.scalar.dma_start(
                            out=y_out[j * P:(j + 1) * P, nb * NBS:(nb + 1) * NBS],
                            in_=osb[:],
                        )
                else:
                    for kt in range(KT):
                        for nb in range(NB):
                            nc.tensor.matmul(
                                psums[nb][:], xT[:, kt, :],
                                wqT[:, kt, nb * NBS:(nb + 1) * NBS],
                                start=(kt == 0), stop=(kt == KT - 1),
                            )
                    for nb in range(NB):
                        osb = osbp.tile([P, NBS], F32, tag="osb", name=f"osb_{j}_{nb}")
                        nc.scalar.activation(
                            osb[:], psums[nb][:],
                            mybir.ActivationFunctionType.Copy, 0.0, scal[:, 2:3],
                        )
                        nc.vector.tensor_tensor(
                            osb[:], osb[:], bias_sb[:, nb * NBS:(nb + 1) * NBS],
                            mybir.AluOpType.add,
                        )
                        nc.scalar.dma_start(
                            out=y_out[j * P:(j + 1) * P, nb * NBS:(nb + 1) * NBS],
                            in_=osb[:],
                        )

    nc.compile()
    return nc


def _compute_gamma(weight: np.ndarray) -> np.float32:
    """Replicate the module's gamma computation bit-exactly (jnp, fp32)."""
    import jax
    import jax.numpy as jnp

    with jax.default_device(jax.devices("cpu")[0]):
        w_f32 = jnp.clip(jnp.asarray(weight, dtype=jnp.float32), -2.0, 2.0)
        gamma = jnp.maximum(jnp.mean(jnp.abs(w_f32)), 1e-4)
        return np.float32(np.asarray(gamma))


def kernel(x: np.ndarray, weight: np.ndarray, bias: np.ndarray) -> np.ndarray:
    global _NC_CACHE, LAST_RESULTS

    x2d = np.ascontiguousarray(np.asarray(x, dtype=np.float32).reshape(M, K))
    weight = np.ascontiguousarray(np.asarray(weight, dtype=np.float32))
    bias = np.asarray(bias, dtype=np.float32)

    gamma = _compute_gamma(weight)
    thr = np.float32(np.float32(0.5) * gamma)
    scal = np.zeros((P, 4), dtype=np.float32)
    scal[:, 0] = thr
    scal[:, 1] = -thr
    scal[:, 2] = np.float32(np.float32(0.5) * gamma)  # psum carries 2x ternary

    if _NC_CACHE is None:
        _NC_CACHE = _build_nc()
    nc = _NC_CACHE

    in_maps = []
    for i in range(N_CORES):
        # host-side layout change only: shard along out_features, then
        # transpose to [K, NS] so device quantization is purely elementwise
        w_shard_T = np.ascontiguousarray(weight[i * NS:(i + 1) * NS].T)
        b_shard = np.ascontiguousarray(
            np.broadcast_to(bias[i * NS:(i + 1) * NS], (P, NS))
        )
        in_maps.append({"x": x2d, "w": w_shard_T, "bias": b_shard, "scal": scal})

    res = run_bass_kernel_spmd(nc, in_maps, list(range(N_CORES)))
    LAST_RESULTS = res

    out = np.concatenate([res.results[i]["out"] for i in range(N_CORES)], axis=1)
    return np.ascontiguousarray(out.reshape(B, S, D_OUT))


# revision 17
# speedup vs baseline: 1.1918x; 1.0496x over previous
"""BitLinear (ternary-quantized linear) Trainium2 kernel.

out = x @ (gamma * ternary(weight)).T + bias, computed tensor-parallel over
8 NeuronCores: weight/bias sharded along out_features, x replicated.

Per-core device program (v5 — no PE transposes, supply-first queue order):
  1. Cast x (fp32) -> bf16 into DRAM scratch via SWDGE casting DMAs,
     throttled so the 32MB fp32 weight stream owns HBM during the prologue.
  2. The weight shard arrives HOST-TRANSPOSED as wT [K, NS] fp32 (layout-only
     host change; quantization happens on device in fp32, preserving the
     exact ternary boundary). [128 k, 512 n] chunks stream nb-major, loads
     alternating across both HWDGE queues, and are quantized elementwise to
     doubled ternary {-2,0,2} bf16 straight into the SBUF-resident
     [K-partition, k-subtile, NS] tile wqT:
       even chunks on ACT:  q' = sign(w - thr) + sign(w + thr), add on DVE
       odd  chunks on DVE:  q' = 2*(w >= thr) - 2*(w <= -thr)
     with thr = 0.5*gamma (factor 2 folded into the gamma/2 output scale).
     All supply-side instructions are emitted before any drain so no queue
     ever blocks on a compute-completion semaphore ahead of supply work.
  3. Prologue compute: the PE processes m-tiles 0..3 one 512-wide n-block
     stage at a time, chasing the quantizer chunk-by-chunk (4 PSUM banks per
     stage, so consecutive stages double-buffer). Steady state (tiles
     4..63) is the per-tile kt-outer / nb-inner loop against the fully
     resident wqT; the last tile runs nb-outer so its drains overlap its
     own matmuls.
  4. Drains: prologue stages evict+bias on DVE with outputs on the Sync
     queue (threaded between steady-state transposes); steady tiles evict
     on ACT (Copy activation, per-partition gamma/2 scale), bias on DVE,
     outputs on the Scalar queue.

gamma = max(mean(|clip(w, -2, 2)|), 1e-4) is a global scalar over the full
weight; it is computed on host with the same jnp ops the module uses so the
quantization boundary matches bit-exactly, and enters the device kernel as a
[128, 4] scalar input tensor (threshold, -threshold, gamma/2).
"""

import numpy as np

import concourse.bass as bass
import concourse.mybir as mybir
import concourse.tile as tile
from concourse import bacc
from concourse.bass_utils import run_bass_kernel_spmd
from concourse.tile import add_dep_helper

P = 128
B, S, D_IN, D_OUT = 4, 2048, 4096, 16384
M = B * S                 # 8192 tokens
K = D_IN                  # 4096 contraction
N_CORES = 8
NS = D_OUT // N_CORES     # 2048 out-features per core
KT = K // P               # 32 k-subtiles
MT = M // P               # 64 m-tiles
NBS = 512                 # psum bank free size (fp32)
NB = NS // NBS            # 4 psum n-blocks

F32 = mybir.dt.float32
BF16 = mybir.dt.bfloat16

_NC_CACHE = None
LAST_RESULTS = None


def _build_nc():
    nc = bacc.Bacc(None, target_bir_lowering=False, debug=False)

    x_in = nc.declare_dram_parameter("x", [M, K], F32, isOutput=False)
    w_in = nc.declare_dram_parameter("w", [K, NS], F32, isOutput=False)
    b_in = nc.declare_dram_parameter("bias", [P, NS], F32, isOutput=False)
    s_in = nc.declare_dram_parameter("scal", [P, 4], F32, isOutput=False)
    y_out = nc.declare_dram_parameter("out", [M, NS], F32, isOutput=True)

    CAST_AHEAD = 6
    PRO_TILES = 4     # m-tiles processed stage-wise during the prologue

    with tile.TileContext(nc) as tc:
        with (
            tc.tile_pool(name="const", bufs=1) as constp,
            tc.tile_pool(name="w_sb", bufs=6) as wsbp,
            tc.tile_pool(name="qab", bufs=6) as qabp,
            tc.tile_pool(name="xT", bufs=5) as xTp,
            tc.tile_pool(name="osb", bufs=6) as osbp,
            tc.tile_pool(name="psum", bufs=8, space="PSUM") as psump,
            tc.tile_pool(name="dram", bufs=1, space="DRAM") as dramp,
        ):
            scal = constp.tile([P, 4], F32)
            nc.sync.dma_start(out=scal[:], in_=s_in[:])
            bias_sb = constp.tile([P, NS], F32)
            nc.sync.dma_start(out=bias_sb[:], in_=b_in[:])
            # full quantized-transposed weight shard, resident in SBUF
            wqT = constp.tile([P, KT, NS], BF16)

            # ---- x fp32 -> bf16 cast, DRAM->DRAM on SWDGE ----
            xhat = []
            cast_insts = []
            for j in range(MT):
                xh = dramp.tile([P, K], BF16, name=f"xhat_{j}")
                if j < PRO_TILES:
                    # sliver the first tiles so each transpose can start as
                    # soon as possible behind the serial SWDGE stream
                    ci = None
                    for s in range(4):
                        r0, r1 = s * 32, (s + 1) * 32
                        ci = nc.gpsimd.dma_start(
                            out=xh[r0:r1, :], in_=x_in[j * P + r0:j * P + r1, :]
                        )
                else:
                    ci = nc.gpsimd.dma_start(
                        out=xh[:], in_=x_in[j * P:(j + 1) * P, :]
                    )
                xhat.append(xh)
                cast_insts.append(ci)

            def emit_xread(j, xT):
                if j < 1:
                    xr = None
                    for s in range(4):
                        r0, r1 = s * 32, (s + 1) * 32
                        xr = nc.sync.dma_start_transpose(
                            xT[:, :, r0:r1], xhat[j][r0:r1, :]
                        )
                else:
                    xr = nc.sync.dma_start_transpose(xT[:], xhat[j][:])
                if j + CAST_AHEAD < MT:
                    add_dep_helper(
                        cast_insts[j + CAST_AHEAD].ins,
                        xr.ins,
                        reason="throttle x-cast to stay a few m-tiles ahead",
                    )
                return xr

            pro_xTs = [
                xTp.tile([P, KT, P], BF16, tag="xT", name=f"xT_{j}")
                for j in range(PRO_TILES)
            ]

            def emit_wload(nb, kt, cidx):
                w_sb = wsbp.tile([P, NBS], F32, tag="w_in")
                dq = nc.sync if (cidx % 2 == 0) else nc.scalar
                wl = dq.dma_start(
                    out=w_sb[:],
                    in_=w_in[kt * P:(kt + 1) * P, nb * NBS:(nb + 1) * NBS],
                )
                return w_sb, wl

            def emit_quant(nb, kt, cidx, w_sb):
                if cidx % 2 == 0:
                    sa = qabp.tile([P, NBS], BF16, tag="q")
                    sb = qabp.tile([P, NBS], BF16, tag="q")
                    nc.scalar.sign(sa[:], w_sb[:], bias=scal[:, 1:2])  # -thr
                    nc.scalar.sign(sb[:], w_sb[:], bias=scal[:, 0:1])  # +thr
                    nc.vector.tensor_tensor(
                        wqT[:, kt, nb * NBS:(nb + 1) * NBS],
                        sa[:], sb[:], mybir.AluOpType.add,
                    )
                else:
                    ga = qabp.tile([P, NBS], BF16, tag="q")
                    gb = qabp.tile([P, NBS], BF16, tag="q")
                    nc.vector.tensor_scalar(
                        ga[:], w_sb[:], scal[:, 0:1], 2.0,
                        mybir.AluOpType.is_ge, mybir.AluOpType.mult,
                    )
                    nc.vector.tensor_scalar(
                        gb[:], w_sb[:], scal[:, 1:2], 2.0,
                        mybir.AluOpType.is_le, mybir.AluOpType.mult,
                    )
                    nc.vector.tensor_tensor(
                        wqT[:, kt, nb * NBS:(nb + 1) * NBS],
                        ga[:], gb[:], mybir.AluOpType.subtract,
                    )

            # ---- supply emission: all 128 weight chunks, nb-major ----
            # x transposes for prologue tiles are threaded between chunk
            # groups at the points they're needed.
            for nb in range(NB):
                for kt in range(KT):
                    cidx = nb * KT + kt
                    w_sb, wl = emit_wload(nb, kt, cidx)
                    emit_quant(nb, kt, cidx, w_sb)
                    if cidx == 7:
                        emit_xread(0, pro_xTs[0])
                    elif cidx == 31:
                        emit_xread(1, pro_xTs[1])
                    elif cidx == 47:
                        emit_xread(2, pro_xTs[2])
                    elif cidx == 63:
                        emit_xread(3, pro_xTs[3])

            # ---- prologue stages: tiles 0..3 against each n-block ----
            pro_drains = []
            for nb in range(NB):
                pss = [
                    psump.tile([P, NBS], F32, tag="ps", name=f"ps_{j}_{nb}")
                    for j in range(PRO_TILES)
                ]
                if nb == 0:
                    # j-outer: later tiles' x is still landing
                    order = [(j, kt) for j in range(PRO_TILES) for kt in range(KT)]
                else:
                    order = [(j, kt) for kt in range(KT) for j in range(PRO_TILES)]
                for j, kt in order:
                    nc.tensor.matmul(
                        pss[j][:],
                        pro_xTs[j][:, kt, :],
                        wqT[:, kt, nb * NBS:(nb + 1) * NBS],
                        start=(kt == 0),
                        stop=(kt == KT - 1),
                    )
                # evict + bias on DVE (keeps the ACT sign stream unblocked);
                # the output DMAs are threaded onto the Sync queue later
                stage_osbs = []
                for j in range(PRO_TILES):
                    osb = osbp.tile([P, NBS], F32, tag="osb", name=f"osb_{j}_{nb}")
                    nc.vector.tensor_scalar(
                        osb[:], pss[j][:], scal[:, 2:3], None,
                        mybir.AluOpType.mult,
                    )
                    nc.vector.tensor_tensor(
                        osb[:], osb[:], bias_sb[:, nb * NBS:(nb + 1) * NBS],
                        mybir.AluOpType.add,
                    )
                    stage_osbs.append((j, osb))
                pro_drains.append(stage_osbs)

            def emit_pro_outs(nb):
                for j, osb in pro_drains[nb]:
                    nc.sync.dma_start(
                        out=y_out[j * P:(j + 1) * P, nb * NBS:(nb + 1) * NBS],
                        in_=osb[:],
                    )

            # ---- steady state: per-tile kt-outer / nb-inner ----
            for j in range(PRO_TILES, MT):
                xT = xTp.tile([P, KT, P], BF16, tag="xT", name=f"xT_{j}")
                # thread the prologue output DMAs between steady transposes
                if j - PRO_TILES < NB:
                    emit_pro_outs(j - PRO_TILES)
                emit_xread(j, xT)
                psums = [
                    psump.tile([P, NBS], F32, tag="ps", name=f"ps_{j}_{nb}")
                    for nb in range(NB)
                ]
                last = j == MT - 1
                if last:
                    # nb-outer: drains overlap this tile's own matmuls
                    for nb in range(NB):
                        for kt in range(KT):
                            nc.tensor.matmul(
                                psums[nb][:], xT[:, kt, :],
                                wqT[:, kt, nb * NBS:(nb + 1) * NBS],
                                start=(kt == 0), stop=(kt == KT - 1),
                            )
                        osb = osbp.tile([P, NBS], F32, tag="osb", name=f"osb_{j}_{nb}")
                        nc.scalar.activation(
                            osb[:], psums[nb][:],
                            mybir.ActivationFunctionType.Copy, 0.0, scal[:, 2:3],
                        )
                        nc.vector.tensor_tensor(
                            osb[:], osb[:], bias_sb[:, nb * NBS:(nb + 1) * NBS],
                            mybir.AluOpType.add,
                        )
                        nc.scalar.dma_start(
                            out=y_out[j * P:(j + 1) * P, nb * NBS:(nb + 1) * NBS],
                            in_=osb[:],
                        )
                else:
                    for kt in range(KT):
                        for nb in range(NB):
                            nc.tensor.matmul(
                                psums[nb][:], xT[:, kt, :],
                                wqT[:, kt, nb * NBS:(nb + 1) * NBS],
                                start=(kt == 0), stop=(kt == KT - 1),
                            )
                    for nb in range(NB):
                        osb = osbp.tile([P, NBS], F32, tag="osb", name=f"osb_{j}_{nb}")
                        nc.scalar.activation(
                            osb[:], psums[nb][:],
                            mybir.ActivationFunctionType.Copy, 0.0, scal[:, 2:3],
                        )
                        nc.vector.tensor_tensor(
                            osb[:], osb[:], bias_sb[:, nb * NBS:(nb + 1) * NBS],
                            mybir.AluOpType.add,
                        )
                        nc.scalar.dma_start(
                            out=y_out[j * P:(j + 1) * P, nb * NBS:(nb + 1) * NBS],
                            in_=osb[:],
                        )

    nc.compile()
    return nc


def _compute_gamma(weight: np.ndarray) -> np.float32:
    """Replicate the module's gamma computation bit-exactly (jnp, fp32)."""
    import jax
    import jax.numpy as jnp

    with jax.default_device(jax.devices("cpu")[0]):
        w_f32 = jnp.clip(jnp.asarray(weight, dtype=jnp.float32), -2.0, 2.0)
        gamma = jnp.maximum(jnp.mean(jnp.abs(w_f32)), 1e-4)
        return np.float32(np.asarray(gamma))


def kernel(x: np.ndarray, weight: np.ndarray, bias: np.ndarray) -> np.ndarray:
    global _NC_CACHE, LAST_RESULTS

    x2d = np.ascontiguousarray(np.asarray(x, dtype=np.float32).reshape(M, K))
    weight = np.ascontiguousarray(np.asarray(weight, dtype=np.float32))
    bias = np.asarray(bias, dtype=np.float32)

    gamma = _compute_gamma(weight)
    thr = np.float32(np.float32(0.5) * gamma)
    scal = np.zeros((P, 4), dtype=np.float32)
    scal[:, 0] = thr
    scal[:, 1] = -thr
    scal[:, 2] = np.float32(np.float32(0.5) * gamma)  # psum carries 2x ternary

    if _NC_CACHE is None:
        _NC_CACHE = _build_nc()
    nc = _NC_CACHE

    in_maps = []
    for i in range(N_CORES):
        # host-side layout change only: shard along out_features, then
        # transpose to [K, NS] so device quantization is purely elementwise
        w_shard_T = np.ascontiguousarray(weight[i * NS:(i + 1) * NS].T)
        b_shard = np.ascontiguousarray(
            np.broadcast_to(bias[i * NS:(i + 1) * NS], (P, NS))
        )
        in_maps.append({"x": x2d, "w": w_shard_T, "bias": b_shard, "scal": scal})

    res = run_bass_kernel_spmd(nc, in_maps, list(range(N_CORES)))
    LAST_RESULTS = res

    out = np.concatenate([res.results[i]["out"] for i in range(N_CORES)], axis=1)
    return np.ascontiguousarray(out.reshape(B, S, D_OUT))
